# revision 2
# baseline (speedup 1.0000x reference)
"""Trainium2 Bass kernel for nn_Decoder_TRANSFORMER_14791867367496.

The reference decoder is affine in the positions: each frame step is
    pos_{t+1} = pos_t @ M + (d_t[b] + g[b,j]),   M = I + W_pe @ W3  (3x3)
(with W_final = [W1; W2; W3] split along its 768 input rows), so the whole
60-step scan has a closed form

    out[b, j, :, t] = X[b, j, :] @ Q_t + r_t[b, :]

where X = initial_grid,
    Q_t = M^t + (W_pe @ W2) @ S_t,          S_t = sum_{k<t} M^k
    r_t[b] = h @ S_t + D_t[b],              D_t = sum_{s=1..t} d_s M^{t-s}
    d_t[b] = (emb_table[t] + z @ W_clip + b_clip) @ W1
    h      = b_pe @ (W2 + W3) + b_final

All of Q/r are tiny (3x3 / per-batch 3-vectors) and are computed on the host
in float64.  The device kernel is then a single affine map per point
([3 feats + bias] -> 180 outputs) and is purely output-bandwidth bound
(94 MB of f32 output; measured per-core DMA saturation ~422 GB/s).

Precision trick: fp32 operands are split into two bf16 chunks
(x = x0+x1, 8 mantissa bits each) and the cross terms with a+b <= 1 are
summed IN A SINGLE MATMUL by stacking them along the contraction dim:
rows [x0 x0 x1] paired against [q0 q1 q0] per feature, plus two ones-rows
paired against the two bf16 chunks of the per-batch bias r.  bf16 products
are exact in fp32; dropped second-order terms are ~2^-18 (measured rel err
2.4e-6 vs the 2e-4 gate).  Per point-pair-tile the K-stack is 11 rows x 2
tiles = K=22, N=2*180=360 (block-diagonal rhs).

Sharding: data-parallel over batch - each of the 8 cores handles 4 batches
(16384 points = 128 point-tiles = 64 packed matmuls).  Output streams out
in fully-linear ~1.47 MB DMAs (the first group goes out as eighth/eighth/
quarter/half so the output stream starts right after matmul 0).

Ramp-up design (the steady-state stream is DMA-saturated, so exec time =
stream time + ramp latency): ALL input DMAs are fully linear in DRAM
(cheap descriptor-gen) and are triggered on the SP ring ahead of the
output triggers: a tiny xt0 (2 matmuls) + rhs0 (local batch 0) unblock
matmul 0 immediately; xt1/rhs1 carry the rest.  The ACT ring issues no
input DMAs so its mandatory activation-table load hides behind the ramp.
PE runs the matmuls, DVE/ACT alternate PSUM->SBUF copies, SP streams the
output.  The device program is raw Bacc with hand-rolled per-edge
semaphores.
"""

import numpy as np

BS, NFRAMES, NJOINTS, NFEATS, LATENT, CLIP = 32, 60, 4096, 3, 256, 512
NCORES = 8
B_PER_CORE = BS // NCORES                  # 4
PTS = B_PER_CORE * NJOINTS                 # 16384 points per core
NTILES = PTS // 128                        # 128 point-tiles per core
GROUPS = 8                                 # output DMA groups
TPG = NTILES // GROUPS                     # 16 tiles per group
FC = NFEATS * NFRAMES                      # 180 output columns per point
KR = 11                                    # K-stack rows per tile (3*3 + 2 bias)
PAIR = 2                                   # tiles fused per matmul
MM_PER_G = TPG // PAIR                     # 8 matmuls per group
NMM = GROUPS * MM_PER_G                    # 64 matmuls per core
XCH = [0, 0, 1]                            # x-chunk index per K row (per feat)
QCH = [0, 1, 0]                            # q-chunk index per K row (per feat)
X0_MM = 2                                  # matmuls covered by the xt0 warm-start


def _split2(a):
    """Split f32 array into two bf16 chunks whose sum reproduces ~16
    mantissa bits.  Returned as f32 arrays holding bf16-representable
    values."""
    import ml_dtypes
    bf = ml_dtypes.bfloat16
    a = np.asarray(a, np.float32)
    a0 = a.astype(bf).astype(np.float32)
    a1 = (a - a0).astype(bf).astype(np.float32)
    return a0, a1


def _precompute(z, W_pe, b_pe, W_clip, b_clip, emb_table, W_final, b_final):
    """Host-side f64 computation of the closed-form coefficients.

    Returns Q_all [3, 180] and r_all [32, 180], column layout c = f*60 + t
    (matching the [.., 3, 60] innermost layout of the output)."""
    f64 = np.float64
    W_pe64 = np.asarray(W_pe, f64)
    W_fin = np.asarray(W_final, f64)
    W1, W2, W3 = W_fin[:LATENT], W_fin[LATENT:2 * LATENT], W_fin[2 * LATENT:]
    M = np.eye(3) + W_pe64 @ W3
    Gm = W_pe64 @ W2
    b_pe64 = np.asarray(b_pe, f64)
    h = b_pe64 @ W2 + b_pe64 @ W3 + np.asarray(b_final, f64)
    z_proj = np.asarray(z, f64) @ np.asarray(W_clip, f64) + np.asarray(b_clip, f64)
    d = (np.asarray(emb_table, f64)[None, :, :] + z_proj[:, None, :]) @ W1  # [32,60,3]

    Q = np.zeros((NFRAMES, 3, 3))
    R = np.zeros((NFRAMES, BS, 3))
    Q[0] = np.eye(3)
    Mt = np.eye(3)
    S = np.zeros((3, 3))
    D = np.zeros((BS, 3))
    for t in range(1, NFRAMES):
        S = S + Mt
        Mt = Mt @ M
        D = D @ M + d[:, t, :]
        Q[t] = Mt + Gm @ S
        R[t] = h @ S + D
    Q_all = Q.transpose(1, 2, 0).reshape(3, FC)     # [k, f*60+t]
    r_all = R.transpose(1, 2, 0).reshape(BS, FC)    # [b, f*60+t]
    return Q_all.astype(np.float32), r_all.astype(np.float32)


N_PS = 8      # psum slots (one bank each; a group cycles all 8)
N_STAGE = 3   # stage buffers


def _copy_seq(j):
    """(engine, 1-based position of copy j within that engine's stream).

    Copies alternate DVE/ACT by matmul index so both engines share every
    group's copy wall."""
    return ("v" if j % 2 == 0 else "a"), j // 2 + 1


def _build_bass():
    import concourse.mybir as mybir
    from concourse import bacc
    from concourse.bass import ts

    f32 = mybir.dt.float32
    bf16 = mybir.dt.bfloat16
    nc = bacc.Bacc(None, target_bir_lowering=False)
    K2 = PAIR * KR
    x0_cols = X0_MM * 128
    xt0 = nc.dram_tensor("xt0", [K2, x0_cols], bf16, kind="ExternalInput")
    xt1 = nc.dram_tensor("xt1", [K2, NMM * 128 - x0_cols], bf16,
                         kind="ExternalInput")
    rhs0 = nc.dram_tensor("rhs0", [K2, PAIR * FC], bf16, kind="ExternalInput")
    rhs1 = nc.dram_tensor("rhs1", [K2, (B_PER_CORE - 1) * PAIR * FC], bf16,
                          kind="ExternalInput")
    out = nc.dram_tensor("out", [PTS, FC], f32, kind="ExternalOutput")
    out_v = out[:].rearrange("(g j w) c -> g j (w c)", g=GROUPS, j=128, w=TPG)

    from contextlib import ExitStack
    ctx = ExitStack()
    xt_sb = ctx.enter_context(nc.sbuf_tensor("xt_sb", [K2, NMM * 128], bf16))
    rhs_sb = ctx.enter_context(
        nc.sbuf_tensor("rhs_sb", [K2, B_PER_CORE * PAIR * FC], bf16))
    stage = [ctx.enter_context(
        nc.sbuf_tensor(f"stage{i}", [128, TPG * FC], f32))
        for i in range(N_STAGE)]
    psum = [ctx.enter_context(
        nc.psum_tensor(f"psum{i}", [128, PAIR * FC], f32))
        for i in range(N_PS)]
    s_x0 = ctx.enter_context(nc.semaphore("s_x0"))
    s_x1 = ctx.enter_context(nc.semaphore("s_x1"))
    s_r0 = ctx.enter_context(nc.semaphore("s_r0"))
    s_r1 = ctx.enter_context(nc.semaphore("s_r1"))
    s_pe = ctx.enter_context(nc.semaphore("s_pe"))
    s_cpv = ctx.enter_context(nc.semaphore("s_cpv"))
    s_cpa = ctx.enter_context(nc.semaphore("s_cpa"))
    s_slot = [ctx.enter_context(nc.semaphore(f"s_slot{i}"))
              for i in range(N_STAGE)]

    # ---- input DMAs: all on the SP ring, all fully linear in DRAM ----
    # xt0+rhs0 are tiny and unblock matmul 0 ASAP; xt1/rhs1 follow and
    # finish well before their first consumers (matmul 2 / group 2).
    nc.sync.dma_start(out=xt_sb[:, :x0_cols], in_=xt0[:]).then_inc(s_x0, 16)
    nc.sync.dma_start(out=rhs_sb[:, :PAIR * FC], in_=rhs0[:]).then_inc(s_r0, 16)
    nc.sync.dma_start(out=xt_sb[:, x0_cols:], in_=xt1[:]).then_inc(s_x1, 16)
    nc.sync.dma_start(out=rhs_sb[:, PAIR * FC:], in_=rhs1[:]).then_inc(s_r1, 16)

    def copies(engine, s_cp_self, g, parity):
        st = stage[g % N_STAGE]
        for sp in range(parity, MM_PER_G, 2):
            j = g * MM_PER_G + sp
            if sp == parity and g >= N_STAGE:
                # stage slot reuse: wait for every out-DMA that read it
                engine.wait_ge(s_slot[g % N_STAGE],
                               16 * slot_reads_before[g])
            engine.wait_ge(s_pe, j + 1)
            if parity == 0:
                nc.vector.tensor_copy(
                    out=st[:, ts(sp, PAIR * FC)], in_=psum[j % N_PS][:]
                ).then_inc(s_cp_self, 1)
            else:
                nc.scalar.copy(
                    out=st[:, ts(sp, PAIR * FC)], in_=psum[j % N_PS][:]
                ).then_inc(s_cp_self, 1)

    # number of completed out-DMA incs required on slot g%3 before group g
    # may overwrite it (group 0's stage goes out as four split DMAs)
    dma_count = {0: 4}
    slot_reads_before = {}
    seen = [0] * N_STAGE
    for g in range(GROUPS):
        slot_reads_before[g] = seen[g % N_STAGE]
        seen[g % N_STAGE] += dma_count.get(g, 1)

    for g in range(GROUPS):
        copies(nc.scalar, s_cpa, g, 1)

    # ---- DVE: even-slot copies ----
    for g in range(GROUPS):
        copies(nc.vector, s_cpv, g, 0)

    # ---- PE: matmuls ----
    for j in range(NMM):
        g, sp = j // MM_PER_G, j % MM_PER_G
        lb = g // 2
        if j == 0:
            nc.tensor.wait_ge(s_x0, 16)
            nc.tensor.wait_ge(s_r0, 16)
        elif j == X0_MM:
            nc.tensor.wait_ge(s_x1, 16)
        if lb == 1 and j % MM_PER_G == 0 and g == 2:
            nc.tensor.wait_ge(s_r1, 16)
        if j >= N_PS:
            # psum slot reuse: wait for the copy that drained it
            eng, pos = _copy_seq(j - N_PS)
            nc.tensor.wait_ge(s_cpv if eng == "v" else s_cpa, pos)
        nc.tensor.matmul(
            psum[j % N_PS][:],
            xt_sb[:, ts(j, 128)],
            rhs_sb[:, ts(lb, PAIR * FC)],
            start=True, stop=True,
        ).then_inc(s_pe, 1)

    # ---- SP: output DMAs ----
    out_v4 = out[:].rearrange("(g j w) c -> g j w c", g=GROUPS, j=128, w=TPG)
    for g in range(GROUPS):
        if g == 0:
            # eighth/eighth/quarter/half DMAs: the stream starts right
            # after matmul 0's copy lands
            for nv, na, w0, w1 in ((1, 0, 0, 2), (1, 1, 2, 4),
                                   (2, 2, 4, 8), (4, 4, 8, TPG)):
                nc.sync.wait_ge(s_cpv, nv)
                if na:
                    nc.sync.wait_ge(s_cpa, na)
                nc.sync.dma_start(
                    out=out_v4[0][:, w0:w1, :],
                    in_=stage[0][:, w0 * FC:w1 * FC],
                ).then_inc(s_slot[0], 16)
            continue
        n_half = MM_PER_G * (g + 1) // 2
        nc.sync.wait_ge(s_cpv, n_half)
        nc.sync.wait_ge(s_cpa, n_half)
        nc.sync.dma_start(out=out_v[g], in_=stage[g % N_STAGE][:]).then_inc(
            s_slot[g % N_STAGE], 16)

    ctx.close()
    nc.finalize()
    return nc


_NC_CACHE = None
_LAST_RESULTS = None  # BassKernelResults of the most recent run (for profiling)


def kernel(z, mask, initial_grid, W_pe, b_pe, W_clip, b_clip, emb_table,
           W_final, b_final):
    global _NC_CACHE, _LAST_RESULTS
    import ml_dtypes
    from concourse import bass_utils

    bf = ml_dtypes.bfloat16
    Q_all, r_all = _precompute(z, W_pe, b_pe, W_clip, b_clip, emb_table,
                               W_final, b_final)
    Qs = _split2(Q_all)                                 # 2 x [3, 180]
    X = np.ascontiguousarray(np.asarray(initial_grid), dtype=np.float32)
    x0_cols = X0_MM * 128

    in_maps = []
    for c in range(NCORES):
        Xc = X[B_PER_CORE * c:B_PER_CORE * (c + 1)].reshape(PTS, NFEATS)
        # point p = g*2048 + j*16 + w lives at tile (g, w), psum partition j
        X4 = Xc.reshape(GROUPS, 128, TPG, NFEATS).transpose(3, 0, 2, 1)
        ch = _split2(X4)                                # 2 x [3, 8, 16, 128]
        A = np.empty((GROUPS, TPG, KR, 128), np.float32)
        for k in range(NFEATS):
            for m in range(3):
                A[:, :, 3 * k + m, :] = ch[XCH[m]][k]
        A[:, :, 9:11, :] = 1.0                          # bias rows
        # matmul s covers tiles (2*(s%8), 2*(s%8)+1) of group s//8;
        # stationary rows 11a.. hold tile a of the pair
        xt_host = (A.reshape(GROUPS, MM_PER_G, PAIR, KR, 128)
                   .transpose(2, 3, 0, 1, 4)
                   .reshape(PAIR * KR, NMM * 128)).astype(bf)

        rhs_host = np.zeros((PAIR * KR, B_PER_CORE * PAIR * FC), np.float32)
        for lb in range(B_PER_CORE):
            rs = _split2(r_all[B_PER_CORE * c + lb])    # 2 x [180]
            R = np.empty((KR, FC), np.float32)
            for k in range(NFEATS):
                for m in range(3):
                    R[3 * k + m] = Qs[QCH[m]][k]
            R[9:11] = np.stack(rs)
            for a in range(PAIR):                       # block-diagonal
                rhs_host[KR * a:KR * (a + 1),
                         lb * PAIR * FC + FC * a: lb * PAIR * FC + FC * (a + 1)] = R
        rhs_host = rhs_host.astype(bf)
        in_maps.append({
            "xt0": np.ascontiguousarray(xt_host[:, :x0_cols]),
            "xt1": np.ascontiguousarray(xt_host[:, x0_cols:]),
            "rhs0": np.ascontiguousarray(rhs_host[:, :PAIR * FC]),
            "rhs1": np.ascontiguousarray(rhs_host[:, PAIR * FC:]),
        })

    if _NC_CACHE is None:
        _NC_CACHE = _build_bass()
    res = bass_utils.run_bass_kernel_spmd(
        _NC_CACHE, in_maps, core_ids=list(range(NCORES))
    )
    _LAST_RESULTS = res

    out = np.empty((BS, NJOINTS, NFEATS, NFRAMES), np.float32)
    for c in range(NCORES):
        out[B_PER_CORE * c:B_PER_CORE * (c + 1)] = (
            res.results[c]["out"].reshape(B_PER_CORE, NJOINTS, NFEATS, NFRAMES)
        )
    return out


# revision 3
# speedup vs baseline: 1.2229x; 1.2229x over previous
"""Trainium2 Bass kernel for nn_Decoder_TRANSFORMER_14791867367496.

The reference decoder is affine in the positions: each frame step is
    pos_{t+1} = pos_t @ M + (d_t[b] + g[b,j]),   M = I + W_pe @ W3  (3x3)
(with W_final = [W1; W2; W3] split along its 768 input rows), so the whole
60-step scan has a closed form

    out[b, j, :, t] = X[b, j, :] @ Q_t + r_t[b, :]

where X = initial_grid,
    Q_t = M^t + (W_pe @ W2) @ S_t,          S_t = sum_{k<t} M^k
    r_t[b] = h @ S_t + D_t[b],              D_t = sum_{s=1..t} d_s M^{t-s}
    d_t[b] = (emb_table[t] + z @ W_clip + b_clip) @ W1
    h      = b_pe @ (W2 + W3) + b_final

All of Q/r are tiny (3x3 / per-batch 3-vectors) and are computed on the host
in float64.  The device kernel is then a single affine map per point
([3 feats + bias] -> 180 outputs) and is purely output-bandwidth bound
(94 MB of f32 output; measured per-core DMA saturation ~422 GB/s).

Precision trick: fp32 operands are split into two bf16 chunks
(x = x0+x1, 8 mantissa bits each) and the cross terms with a+b <= 1 are
summed IN A SINGLE MATMUL by stacking them along the contraction dim:
rows [x0 x0 x1] paired against [q0 q1 q0] per feature, plus two ones-rows
paired against the two bf16 chunks of the per-batch bias r.  bf16 products
are exact in fp32; dropped second-order terms are ~2^-18 (measured rel err
2.4e-6 vs the 2e-4 gate).  Per point-pair-tile the K-stack is 11 rows x 2
tiles = K=22, N=2*180=360 (block-diagonal rhs).

Sharding: data-parallel over batch - each of the 8 cores handles 4 batches
(16384 points = 128 point-tiles = 64 packed matmuls).  Output streams out
in fully-linear ~1.47 MB DMAs (the first group goes out as eighth/eighth/
quarter/half so the output stream starts right after matmul 0).

Ramp/pipeline design (steady state is DMA-saturated, so exec time =
stream time + ramp latency; DMA completion semaphores lag the trigger's
descriptor-gen by a fixed ~1.4 us, and each HWDGE ring serializes
descriptor-gen ~0.6-1.0 us per DMA):
  - SP ring carries ONLY xt0a+rhs0 (the two tiny DMAs matmul 0 needs)
    and then the output DMAs, so the first output descgen is never queued
    behind bulk input.
  - ACT ring carries all bulk input (c0b, rhs1, c1..c7), completion
    tracked by cumulative thresholds on one semaphore (same queue =>
    in-order completion; every DMA increments by exactly 16).
  - Six stage buffers so the PSUM->SBUF copies never wait on the ~1.4 us
    out-DMA completion latency chain.
PE runs the matmuls, DVE/ACT alternate PSUM->SBUF copies, SP streams the
output.  The device program is raw Bacc with hand-rolled per-edge
semaphores.
"""

import numpy as np

BS, NFRAMES, NJOINTS, NFEATS, LATENT, CLIP = 32, 60, 4096, 3, 256, 512
NCORES = 8
B_PER_CORE = BS // NCORES                  # 4
PTS = B_PER_CORE * NJOINTS                 # 16384 points per core
NTILES = PTS // 128                        # 128 point-tiles per core
GROUPS = 8                                 # output DMA groups
TPG = NTILES // GROUPS                     # 16 tiles per group
FC = NFEATS * NFRAMES                      # 180 output columns per point
KR = 11                                    # K-stack rows per tile (3*3 + 2 bias)
PAIR = 2                                   # tiles fused per matmul
MM_PER_G = TPG // PAIR                     # 8 matmuls per group
NMM = GROUPS * MM_PER_G                    # 64 matmuls per core
XCH = [0, 0, 1]                            # x-chunk index per K row (per feat)
QCH = [0, 1, 0]                            # q-chunk index per K row (per feat)
X0_MM = 2                                  # matmuls covered by the xt0a warm-start


def _split2(a):
    """Split f32 array into two bf16 chunks whose sum reproduces ~16
    mantissa bits.  Returned as f32 arrays holding bf16-representable
    values."""
    import ml_dtypes
    bf = ml_dtypes.bfloat16
    a = np.asarray(a, np.float32)
    a0 = a.astype(bf).astype(np.float32)
    a1 = (a - a0).astype(bf).astype(np.float32)
    return a0, a1


def _precompute(z, W_pe, b_pe, W_clip, b_clip, emb_table, W_final, b_final):
    """Host-side f64 computation of the closed-form coefficients.

    Returns Q_all [3, 180] and r_all [32, 180], column layout c = f*60 + t
    (matching the [.., 3, 60] innermost layout of the output)."""
    f64 = np.float64
    W_pe64 = np.asarray(W_pe, f64)
    W_fin = np.asarray(W_final, f64)
    W1, W2, W3 = W_fin[:LATENT], W_fin[LATENT:2 * LATENT], W_fin[2 * LATENT:]
    M = np.eye(3) + W_pe64 @ W3
    Gm = W_pe64 @ W2
    b_pe64 = np.asarray(b_pe, f64)
    h = b_pe64 @ W2 + b_pe64 @ W3 + np.asarray(b_final, f64)
    z_proj = np.asarray(z, f64) @ np.asarray(W_clip, f64) + np.asarray(b_clip, f64)
    d = (np.asarray(emb_table, f64)[None, :, :] + z_proj[:, None, :]) @ W1  # [32,60,3]

    Q = np.zeros((NFRAMES, 3, 3))
    R = np.zeros((NFRAMES, BS, 3))
    Q[0] = np.eye(3)
    Mt = np.eye(3)
    S = np.zeros((3, 3))
    D = np.zeros((BS, 3))
    for t in range(1, NFRAMES):
        S = S + Mt
        Mt = Mt @ M
        D = D @ M + d[:, t, :]
        Q[t] = Mt + Gm @ S
        R[t] = h @ S + D
    Q_all = Q.transpose(1, 2, 0).reshape(3, FC)     # [k, f*60+t]
    r_all = R.transpose(1, 2, 0).reshape(BS, FC)    # [b, f*60+t]
    return Q_all.astype(np.float32), r_all.astype(np.float32)


N_PS = 8      # psum slots (one bank each; a group cycles all 8)
N_STAGE = 6   # stage buffers


def _copy_seq(j):
    """(engine, 1-based position of copy j within that engine's stream).

    Copies alternate DVE/ACT by matmul index so both engines share every
    group's copy wall."""
    return ("v" if j % 2 == 0 else "a"), j // 2 + 1


def _build_bass():
    import concourse.mybir as mybir
    from concourse import bacc
    from concourse.bass import ts

    f32 = mybir.dt.float32
    bf16 = mybir.dt.bfloat16
    nc = bacc.Bacc(None, target_bir_lowering=False)
    K2 = PAIR * KR
    x0_cols = X0_MM * 128
    g_cols = MM_PER_G * 128
    # SP-ring inputs: just enough for matmuls 0..1
    xt0 = nc.dram_tensor("xt0", [K2, x0_cols], bf16, kind="ExternalInput")
    rhs0 = nc.dram_tensor("rhs0", [K2, PAIR * FC], bf16, kind="ExternalInput")
    # ACT-ring inputs: rest of group 0, remaining batches' rhs, groups 1-7
    c0b = nc.dram_tensor("c0b", [K2, g_cols - x0_cols], bf16,
                         kind="ExternalInput")
    rhs1 = nc.dram_tensor("rhs1", [K2, (B_PER_CORE - 1) * PAIR * FC], bf16,
                          kind="ExternalInput")
    cg = [nc.dram_tensor(f"c{g}", [K2, g_cols], bf16, kind="ExternalInput")
          for g in range(1, GROUPS)]
    out = nc.dram_tensor("out", [PTS, FC], f32, kind="ExternalOutput")
    out_v = out[:].rearrange("(g j w) c -> g j (w c)", g=GROUPS, j=128, w=TPG)

    from contextlib import ExitStack
    ctx = ExitStack()
    xt_sb = ctx.enter_context(nc.sbuf_tensor("xt_sb", [K2, NMM * 128], bf16))
    rhs_sb = ctx.enter_context(
        nc.sbuf_tensor("rhs_sb", [K2, B_PER_CORE * PAIR * FC], bf16))
    stage = [ctx.enter_context(
        nc.sbuf_tensor(f"stage{i}", [128, TPG * FC], f32))
        for i in range(N_STAGE)]
    psum = [ctx.enter_context(
        nc.psum_tensor(f"psum{i}", [128, PAIR * FC], f32))
        for i in range(N_PS)]
    s_x0 = ctx.enter_context(nc.semaphore("s_x0"))
    s_r0 = ctx.enter_context(nc.semaphore("s_r0"))
    s_xin = ctx.enter_context(nc.semaphore("s_xin"))
    s_pe = ctx.enter_context(nc.semaphore("s_pe"))
    s_cpv = ctx.enter_context(nc.semaphore("s_cpv"))
    s_cpa = ctx.enter_context(nc.semaphore("s_cpa"))
    s_slot = [ctx.enter_context(nc.semaphore(f"s_slot{i}"))
              for i in range(N_STAGE)]

    # ---- input DMAs ----
    # SP ring: the two tiny warm-start DMAs, then nothing but output.
    nc.sync.dma_start(out=xt_sb[:, :x0_cols], in_=xt0[:]).then_inc(s_x0, 16)
    nc.sync.dma_start(out=rhs_sb[:, :PAIR * FC], in_=rhs0[:]).then_inc(s_r0, 16)
    # ACT ring: bulk input, cumulative completion thresholds on s_xin
    # (position i done <=> s_xin >= 16*(i+1)).
    nc.scalar.dma_start(out=xt_sb[:, x0_cols:g_cols],
                        in_=c0b[:]).then_inc(s_xin, 16)
    nc.scalar.dma_start(out=rhs_sb[:, PAIR * FC:],
                        in_=rhs1[:]).then_inc(s_xin, 16)
    for g in range(1, GROUPS):
        nc.scalar.dma_start(out=xt_sb[:, ts(g, g_cols)],
                            in_=cg[g - 1][:]).then_inc(s_xin, 16)

    def copies(engine, s_cp_self, g, parity):
        st = stage[g % N_STAGE]
        for sp in range(parity, MM_PER_G, 2):
            j = g * MM_PER_G + sp
            if sp == parity and g >= N_STAGE:
                # stage slot reuse: wait for every out-DMA that read it
                engine.wait_ge(s_slot[g % N_STAGE],
                               16 * slot_reads_before[g])
            engine.wait_ge(s_pe, j + 1)
            if parity == 0:
                nc.vector.tensor_copy(
                    out=st[:, ts(sp, PAIR * FC)], in_=psum[j % N_PS][:]
                ).then_inc(s_cp_self, 1)
            else:
                nc.scalar.copy(
                    out=st[:, ts(sp, PAIR * FC)], in_=psum[j % N_PS][:]
                ).then_inc(s_cp_self, 1)

    # number of completed out-DMA incs required on slot g%N_STAGE before
    # group g may overwrite it (group 0's stage goes out as 4 split DMAs)
    dma_count = {0: 4}
    slot_reads_before = {}
    seen = [0] * N_STAGE
    for g in range(GROUPS):
        slot_reads_before[g] = seen[g % N_STAGE]
        seen[g % N_STAGE] += dma_count.get(g, 1)

    for g in range(GROUPS):
        copies(nc.scalar, s_cpa, g, 1)

    # ---- DVE: even-slot copies ----
    for g in range(GROUPS):
        copies(nc.vector, s_cpv, g, 0)

    # ---- PE: matmuls ----
    for j in range(NMM):
        g, sp = j // MM_PER_G, j % MM_PER_G
        lb = g // 2
        if j == 0:
            nc.tensor.wait_ge(s_x0, 16)
            nc.tensor.wait_ge(s_r0, 16)
        elif j == X0_MM:
            nc.tensor.wait_ge(s_xin, 16)          # c0b
        elif j == 2 * MM_PER_G:
            pass                                   # rhs1 covered by c2 wait
        if sp == 0 and g >= 1:
            # chunk g is ACT-queue position g+1 (after c0b, rhs1)
            nc.tensor.wait_ge(s_xin, 16 * (g + 2))
        if j >= N_PS:
            # psum slot reuse: wait for the copy that drained it
            eng, pos = _copy_seq(j - N_PS)
            nc.tensor.wait_ge(s_cpv if eng == "v" else s_cpa, pos)
        nc.tensor.matmul(
            psum[j % N_PS][:],
            xt_sb[:, ts(j, 128)],
            rhs_sb[:, ts(lb, PAIR * FC)],
            start=True, stop=True,
        ).then_inc(s_pe, 1)

    # ---- SP: output DMAs ----
    out_v4 = out[:].rearrange("(g j w) c -> g j w c", g=GROUPS, j=128, w=TPG)
    for g in range(GROUPS):
        if g == 0:
            # eighth/eighth/quarter/half DMAs: the stream starts right
            # after matmul 0's copy lands
            for nv, na, w0, w1 in ((1, 0, 0, 2), (1, 1, 2, 4),
                                   (2, 2, 4, 8), (4, 4, 8, TPG)):
                nc.sync.wait_ge(s_cpv, nv)
                if na:
                    nc.sync.wait_ge(s_cpa, na)
                nc.sync.dma_start(
                    out=out_v4[0][:, w0:w1, :],
                    in_=stage[0][:, w0 * FC:w1 * FC],
                ).then_inc(s_slot[0], 16)
            continue
        n_half = MM_PER_G * (g + 1) // 2
        nc.sync.wait_ge(s_cpv, n_half)
        nc.sync.wait_ge(s_cpa, n_half)
        nc.sync.dma_start(out=out_v[g], in_=stage[g % N_STAGE][:]).then_inc(
            s_slot[g % N_STAGE], 16)

    ctx.close()
    nc.finalize()
    return nc


_NC_CACHE = None
_LAST_RESULTS = None  # BassKernelResults of the most recent run (for profiling)


def kernel(z, mask, initial_grid, W_pe, b_pe, W_clip, b_clip, emb_table,
           W_final, b_final):
    global _NC_CACHE, _LAST_RESULTS
    import ml_dtypes
    from concourse import bass_utils

    bf = ml_dtypes.bfloat16
    Q_all, r_all = _precompute(z, W_pe, b_pe, W_clip, b_clip, emb_table,
                               W_final, b_final)
    Qs = _split2(Q_all)                                 # 2 x [3, 180]
    X = np.ascontiguousarray(np.asarray(initial_grid), dtype=np.float32)
    x0_cols = X0_MM * 128
    g_cols = MM_PER_G * 128

    in_maps = []
    for c in range(NCORES):
        Xc = X[B_PER_CORE * c:B_PER_CORE * (c + 1)].reshape(PTS, NFEATS)
        # point p = g*2048 + j*16 + w lives at tile (g, w), psum partition j
        X4 = Xc.reshape(GROUPS, 128, TPG, NFEATS).transpose(3, 0, 2, 1)
        ch = _split2(X4)                                # 2 x [3, 8, 16, 128]
        A = np.empty((GROUPS, TPG, KR, 128), np.float32)
        for k in range(NFEATS):
            for m in range(3):
                A[:, :, 3 * k + m, :] = ch[XCH[m]][k]
        A[:, :, 9:11, :] = 1.0                          # bias rows
        # matmul s covers tiles (2*(s%8), 2*(s%8)+1) of group s//8;
        # stationary rows 11a.. hold tile a of the pair
        xt_host = (A.reshape(GROUPS, MM_PER_G, PAIR, KR, 128)
                   .transpose(2, 3, 0, 1, 4)
                   .reshape(PAIR * KR, NMM * 128)).astype(bf)

        rhs_host = np.zeros((PAIR * KR, B_PER_CORE * PAIR * FC), np.float32)
        for lb in range(B_PER_CORE):
            rs = _split2(r_all[B_PER_CORE * c + lb])    # 2 x [180]
            R = np.empty((KR, FC), np.float32)
            for k in range(NFEATS):
                for m in range(3):
                    R[3 * k + m] = Qs[QCH[m]][k]
            R[9:11] = np.stack(rs)
            for a in range(PAIR):                       # block-diagonal
                rhs_host[KR * a:KR * (a + 1),
                         lb * PAIR * FC + FC * a: lb * PAIR * FC + FC * (a + 1)] = R
        rhs_host = rhs_host.astype(bf)
        im = {
            "xt0": np.ascontiguousarray(xt_host[:, :x0_cols]),
            "c0b": np.ascontiguousarray(xt_host[:, x0_cols:g_cols]),
            "rhs0": np.ascontiguousarray(rhs_host[:, :PAIR * FC]),
            "rhs1": np.ascontiguousarray(rhs_host[:, PAIR * FC:]),
        }
        for g in range(1, GROUPS):
            im[f"c{g}"] = np.ascontiguousarray(
                xt_host[:, g * g_cols:(g + 1) * g_cols])
        in_maps.append(im)

    if _NC_CACHE is None:
        _NC_CACHE = _build_bass()
    res = bass_utils.run_bass_kernel_spmd(
        _NC_CACHE, in_maps, core_ids=list(range(NCORES))
    )
    _LAST_RESULTS = res

    out = np.empty((BS, NJOINTS, NFEATS, NFRAMES), np.float32)
    for c in range(NCORES):
        out[B_PER_CORE * c:B_PER_CORE * (c + 1)] = (
            res.results[c]["out"].reshape(B_PER_CORE, NJOINTS, NFEATS, NFRAMES)
        )
    return out


# revision 4
# speedup vs baseline: 1.3187x; 1.0783x over previous
"""Trainium2 Bass kernel for nn_Decoder_TRANSFORMER_14791867367496.

The reference decoder is affine in the positions: each frame step is
    pos_{t+1} = pos_t @ M + (d_t[b] + g[b,j]),   M = I + W_pe @ W3  (3x3)
(with W_final = [W1; W2; W3] split along its 768 input rows), so the whole
60-step scan has a closed form

    out[b, j, :, t] = X[b, j, :] @ Q_t + r_t[b, :]

where X = initial_grid,
    Q_t = M^t + (W_pe @ W2) @ S_t,          S_t = sum_{k<t} M^k
    r_t[b] = h @ S_t + D_t[b],              D_t = sum_{s=1..t} d_s M^{t-s}
    d_t[b] = (emb_table[t] + z @ W_clip + b_clip) @ W1
    h      = b_pe @ (W2 + W3) + b_final

All of Q/r are tiny (3x3 / per-batch 3-vectors) and are computed on the host
in float64.  The device kernel is then a single affine map per point
([3 feats + bias] -> 180 outputs) and is purely output-bandwidth bound
(94 MB of f32 output; measured per-core DMA saturation ~422 GB/s).

Precision trick: fp32 operands are split into two bf16 chunks
(x = x0+x1, 8 mantissa bits each) and the cross terms with a+b <= 1 are
summed IN A SINGLE MATMUL by stacking them along the contraction dim:
rows [x0 x0 x1] paired against [q0 q1 q0] per feature, plus two ones-rows
paired against the two bf16 chunks of the per-batch bias r.  bf16 products
are exact in fp32; dropped second-order terms are ~2^-18 (measured rel err
2.4e-6 vs the 2e-4 gate).  Per point-pair-tile the K-stack is 11 rows x 2
tiles = K=22, N=2*180=360 (block-diagonal rhs).

Sharding: data-parallel over batch - each of the 8 cores handles 4 batches
(16384 points = 128 point-tiles = 64 packed matmuls).  Output streams out
in fully-linear ~1.47 MB DMAs, the first group split 1/1/2/2/2 matmuls so
the stream starts right after matmul 0's copy.

Ramp/pipeline design (steady state is DMA-saturated, so exec time =
stream time + ramp latency; DMA completion semaphores lag the trigger's
descriptor-gen by a fixed ~1.4 us, descriptor-gen blocks the issuing
engine ~0.6-1.0 us per DMA, and all SBUF operands live in ONE sbuf tensor
so multi-part input loads stay fully linear):
  - SP ring carries ONE tiny head DMA (matmul 0's stationary + rhs
    columns) and then only the output DMAs.
  - ACT ring carries the bulk input as three linear DMAs (rest of group
    0; group 1; groups 2-7 + remaining rhs), completion tracked by
    cumulative thresholds on one semaphore (same queue => in-order
    completion; every DMA increments by exactly 16).  The ACT engine is
    free again by the time its first PSUM copy is due.
  - Six stage buffers so the PSUM->SBUF copies never wait on the ~1.4 us
    out-DMA completion latency chain.
PE runs the matmuls, DVE/ACT alternate PSUM->SBUF copies, SP streams the
output.  The device program is raw Bacc with hand-rolled per-edge
semaphores.
"""

import numpy as np

BS, NFRAMES, NJOINTS, NFEATS, LATENT, CLIP = 32, 60, 4096, 3, 256, 512
NCORES = 8
B_PER_CORE = BS // NCORES                  # 4
PTS = B_PER_CORE * NJOINTS                 # 16384 points per core
NTILES = PTS // 128                        # 128 point-tiles per core
GROUPS = 8                                 # output DMA groups
TPG = NTILES // GROUPS                     # 16 tiles per group
FC = NFEATS * NFRAMES                      # 180 output columns per point
KR = 11                                    # K-stack rows per tile (3*3 + 2 bias)
PAIR = 2                                   # tiles fused per matmul
MM_PER_G = TPG // PAIR                     # 8 matmuls per group
NMM = GROUPS * MM_PER_G                    # 64 matmuls per core
XCH = [0, 0, 1]                            # x-chunk index per K row (per feat)
QCH = [0, 1, 0]                            # q-chunk index per K row (per feat)
K2 = PAIR * KR                             # matmul contraction dim (22)
XT_COLS = NMM * 128                        # 8192 stationary columns
RHS_COLS = B_PER_CORE * PAIR * FC          # 1440 moving columns
HEAD_X = 2 * 128                           # stationary cols in the head DMA
HEAD = HEAD_X + PAIR * FC                  # head DMA cols (616)
W_COLS = XT_COLS + RHS_COLS                # single sbuf tensor width (9632)
# group-0 output split: matmuls per DMA (first DMA fires after copy 0)
G0_SPLIT = (1, 1, 2, 2, 2)


def _xt_col(c):
    """wsb column of stationary (xt) column c."""
    return c if c < HEAD_X else PAIR * FC + c


def _rhs_col(r):
    """wsb column of moving (rhs) column r."""
    return HEAD_X + r if r < PAIR * FC else XT_COLS + r


def _split2(a):
    """Split f32 array into two bf16 chunks whose sum reproduces ~16
    mantissa bits.  Returned as f32 arrays holding bf16-representable
    values."""
    import ml_dtypes
    bf = ml_dtypes.bfloat16
    a = np.asarray(a, np.float32)
    a0 = a.astype(bf).astype(np.float32)
    a1 = (a - a0).astype(bf).astype(np.float32)
    return a0, a1


def _precompute(z, W_pe, b_pe, W_clip, b_clip, emb_table, W_final, b_final):
    """Host-side f64 computation of the closed-form coefficients.

    Returns Q_all [3, 180] and r_all [32, 180], column layout c = f*60 + t
    (matching the [.., 3, 60] innermost layout of the output)."""
    f64 = np.float64
    W_pe64 = np.asarray(W_pe, f64)
    W_fin = np.asarray(W_final, f64)
    W1, W2, W3 = W_fin[:LATENT], W_fin[LATENT:2 * LATENT], W_fin[2 * LATENT:]
    M = np.eye(3) + W_pe64 @ W3
    Gm = W_pe64 @ W2
    b_pe64 = np.asarray(b_pe, f64)
    h = b_pe64 @ W2 + b_pe64 @ W3 + np.asarray(b_final, f64)
    z_proj = np.asarray(z, f64) @ np.asarray(W_clip, f64) + np.asarray(b_clip, f64)
    d = (np.asarray(emb_table, f64)[None, :, :] + z_proj[:, None, :]) @ W1  # [32,60,3]

    Q = np.zeros((NFRAMES, 3, 3))
    R = np.zeros((NFRAMES, BS, 3))
    Q[0] = np.eye(3)
    Mt = np.eye(3)
    S = np.zeros((3, 3))
    D = np.zeros((BS, 3))
    for t in range(1, NFRAMES):
        S = S + Mt
        Mt = Mt @ M
        D = D @ M + d[:, t, :]
        Q[t] = Mt + Gm @ S
        R[t] = h @ S + D
    Q_all = Q.transpose(1, 2, 0).reshape(3, FC)     # [k, f*60+t]
    r_all = R.transpose(1, 2, 0).reshape(BS, FC)    # [b, f*60+t]
    return Q_all.astype(np.float32), r_all.astype(np.float32)


N_PS = 8      # psum slots (one bank each; a group cycles all 8)
N_STAGE = 6   # stage buffers


def _copy_seq(j):
    """(engine, 1-based position of copy j within that engine's stream).

    Copies alternate DVE/ACT by matmul index so both engines share every
    group's copy wall."""
    return ("v" if j % 2 == 0 else "a"), j // 2 + 1


def _build_bass():
    import concourse.mybir as mybir
    from concourse import bacc
    from concourse.bass import ts

    f32 = mybir.dt.float32
    bf16 = mybir.dt.bfloat16
    nc = bacc.Bacc(None, target_bir_lowering=False)
    # bulk boundaries in xt-column space: rest of group 0 | group 1 |
    # groups 2-7 (the last bulk also carries the remaining rhs columns)
    b1_cols = MM_PER_G * 128 - HEAD_X
    b2_cols = MM_PER_G * 128
    b3_cols = (GROUPS - 2) * MM_PER_G * 128 + (B_PER_CORE - 1) * PAIR * FC
    head = nc.dram_tensor("head", [K2, HEAD], bf16, kind="ExternalInput")
    bulk1 = nc.dram_tensor("bulk1", [K2, b1_cols], bf16, kind="ExternalInput")
    bulk2 = nc.dram_tensor("bulk2", [K2, b2_cols], bf16, kind="ExternalInput")
    bulk3 = nc.dram_tensor("bulk3", [K2, b3_cols], bf16, kind="ExternalInput")
    out = nc.dram_tensor("out", [PTS, FC], f32, kind="ExternalOutput")
    out_v = out[:].rearrange("(g j w) c -> g j (w c)", g=GROUPS, j=128, w=TPG)

    from contextlib import ExitStack
    ctx = ExitStack()
    wsb = ctx.enter_context(nc.sbuf_tensor("wsb", [K2, W_COLS], bf16))
    stage = [ctx.enter_context(
        nc.sbuf_tensor(f"stage{i}", [128, TPG * FC], f32))
        for i in range(N_STAGE)]
    psum = [ctx.enter_context(
        nc.psum_tensor(f"psum{i}", [128, PAIR * FC], f32))
        for i in range(N_PS)]
    s_head = ctx.enter_context(nc.semaphore("s_head"))
    s_xin = ctx.enter_context(nc.semaphore("s_xin"))
    s_pe = ctx.enter_context(nc.semaphore("s_pe"))
    s_cpv = ctx.enter_context(nc.semaphore("s_cpv"))
    s_cpa = ctx.enter_context(nc.semaphore("s_cpa"))
    s_slot = [ctx.enter_context(nc.semaphore(f"s_slot{i}"))
              for i in range(N_STAGE)]

    # ---- input DMAs ----
    # SP ring: one tiny head DMA (matmul 0+1 stationary cols + local batch
    # 0's rhs), then nothing but output.
    nc.sync.dma_start(out=wsb[:, :HEAD], in_=head[:]).then_inc(s_head, 16)
    # ACT ring: bulk input, cumulative completion thresholds on s_xin
    # (position i done <=> s_xin >= 16*(i+1)).
    c0 = HEAD
    for t, w in ((bulk1, b1_cols), (bulk2, b2_cols), (bulk3, b3_cols)):
        nc.scalar.dma_start(out=wsb[:, c0:c0 + w], in_=t[:]).then_inc(s_xin, 16)
        c0 += w

    def copies(engine, s_cp_self, g, parity):
        st = stage[g % N_STAGE]
        for sp in range(parity, MM_PER_G, 2):
            j = g * MM_PER_G + sp
            if sp == parity and g >= N_STAGE:
                # stage slot reuse: wait for every out-DMA that read it
                engine.wait_ge(s_slot[g % N_STAGE],
                               16 * slot_reads_before[g])
            engine.wait_ge(s_pe, j + 1)
            if parity == 0:
                nc.vector.tensor_copy(
                    out=st[:, ts(sp, PAIR * FC)], in_=psum[j % N_PS][:]
                ).then_inc(s_cp_self, 1)
            else:
                nc.scalar.copy(
                    out=st[:, ts(sp, PAIR * FC)], in_=psum[j % N_PS][:]
                ).then_inc(s_cp_self, 1)

    # number of completed out-DMA incs required on slot g%N_STAGE before
    # group g may overwrite it (group 0's stage goes out as split DMAs)
    dma_count = {0: len(G0_SPLIT)}
    slot_reads_before = {}
    seen = [0] * N_STAGE
    for g in range(GROUPS):
        slot_reads_before[g] = seen[g % N_STAGE]
        seen[g % N_STAGE] += dma_count.get(g, 1)

    for g in range(GROUPS):
        copies(nc.scalar, s_cpa, g, 1)

    # ---- DVE: even-slot copies ----
    for g in range(GROUPS):
        copies(nc.vector, s_cpv, g, 0)

    # ---- PE: matmuls ----
    for j in range(NMM):
        g, sp = j // MM_PER_G, j % MM_PER_G
        lb = g // 2
        if j == 0:
            nc.tensor.wait_ge(s_head, 16)
        elif j == 2:
            nc.tensor.wait_ge(s_xin, 16)          # bulk1: rest of group 0
        elif j == MM_PER_G:
            nc.tensor.wait_ge(s_xin, 32)          # bulk2: group 1
        elif j == 2 * MM_PER_G:
            nc.tensor.wait_ge(s_xin, 48)          # bulk3: groups 2-7 + rhs
        if j >= N_PS:
            # psum slot reuse: wait for the copy that drained it
            eng, pos = _copy_seq(j - N_PS)
            nc.tensor.wait_ge(s_cpv if eng == "v" else s_cpa, pos)
        nc.tensor.matmul(
            psum[j % N_PS][:],
            wsb[:, _xt_col(j * 128):_xt_col(j * 128) + 128],
            wsb[:, _rhs_col(lb * PAIR * FC):_rhs_col(lb * PAIR * FC) + PAIR * FC],
            start=True, stop=True,
        ).then_inc(s_pe, 1)

    # ---- SP: output DMAs ----
    out_v4 = out[:].rearrange("(g j w) c -> g j w c", g=GROUPS, j=128, w=TPG)
    for g in range(GROUPS):
        if g == 0:
            # split DMAs: the stream starts right after matmul 0's copy
            mm0 = 0
            for nmm in G0_SPLIT:
                w0, w1 = PAIR * mm0, PAIR * (mm0 + nmm)
                last = mm0 + nmm - 1            # last matmul this DMA needs
                nc.sync.wait_ge(s_cpv, last // 2 + 1)
                if last >= 1:
                    nc.sync.wait_ge(s_cpa, (last - 1) // 2 + 1)
                nc.sync.dma_start(
                    out=out_v4[0][:, w0:w1, :],
                    in_=stage[0][:, w0 * FC:w1 * FC],
                ).then_inc(s_slot[0], 16)
                mm0 += nmm
            continue
        n_half = MM_PER_G * (g + 1) // 2
        nc.sync.wait_ge(s_cpv, n_half)
        nc.sync.wait_ge(s_cpa, n_half)
        nc.sync.dma_start(out=out_v[g], in_=stage[g % N_STAGE][:]).then_inc(
            s_slot[g % N_STAGE], 16)

    ctx.close()
    nc.finalize()
    return nc


_NC_CACHE = None
_LAST_RESULTS = None  # BassKernelResults of the most recent run (for profiling)


def kernel(z, mask, initial_grid, W_pe, b_pe, W_clip, b_clip, emb_table,
           W_final, b_final):
    global _NC_CACHE, _LAST_RESULTS
    import ml_dtypes
    from concourse import bass_utils

    bf = ml_dtypes.bfloat16
    Q_all, r_all = _precompute(z, W_pe, b_pe, W_clip, b_clip, emb_table,
                               W_final, b_final)
    Qs = _split2(Q_all)                                 # 2 x [3, 180]
    X = np.ascontiguousarray(np.asarray(initial_grid), dtype=np.float32)
    g_cols = MM_PER_G * 128

    in_maps = []
    for c in range(NCORES):
        Xc = X[B_PER_CORE * c:B_PER_CORE * (c + 1)].reshape(PTS, NFEATS)
        # point p = g*2048 + j*16 + w lives at tile (g, w), psum partition j
        X4 = Xc.reshape(GROUPS, 128, TPG, NFEATS).transpose(3, 0, 2, 1)
        ch = _split2(X4)                                # 2 x [3, 8, 16, 128]
        A = np.empty((GROUPS, TPG, KR, 128), np.float32)
        for k in range(NFEATS):
            for m in range(3):
                A[:, :, 3 * k + m, :] = ch[XCH[m]][k]
        A[:, :, 9:11, :] = 1.0                          # bias rows
        # matmul s covers tiles (2*(s%8), 2*(s%8)+1) of group s//8;
        # stationary rows 11a.. hold tile a of the pair
        xt_host = (A.reshape(GROUPS, MM_PER_G, PAIR, KR, 128)
                   .transpose(2, 3, 0, 1, 4)
                   .reshape(K2, XT_COLS))

        rhs_host = np.zeros((K2, RHS_COLS), np.float32)
        for lb in range(B_PER_CORE):
            rs = _split2(r_all[B_PER_CORE * c + lb])    # 2 x [180]
            R = np.empty((KR, FC), np.float32)
            for k in range(NFEATS):
                for m in range(3):
                    R[3 * k + m] = Qs[QCH[m]][k]
            R[9:11] = np.stack(rs)
            for a in range(PAIR):                       # block-diagonal
                rhs_host[KR * a:KR * (a + 1),
                         lb * PAIR * FC + FC * a: lb * PAIR * FC + FC * (a + 1)] = R
        xt_host = xt_host.astype(bf)
        rhs_host = rhs_host.astype(bf)
        in_maps.append({
            "head": np.ascontiguousarray(
                np.concatenate([xt_host[:, :HEAD_X],
                                rhs_host[:, :PAIR * FC]], axis=1)),
            "bulk1": np.ascontiguousarray(xt_host[:, HEAD_X:g_cols]),
            "bulk2": np.ascontiguousarray(xt_host[:, g_cols:2 * g_cols]),
            "bulk3": np.ascontiguousarray(
                np.concatenate([xt_host[:, 2 * g_cols:],
                                rhs_host[:, PAIR * FC:]], axis=1)),
        })

    if _NC_CACHE is None:
        _NC_CACHE = _build_bass()
    res = bass_utils.run_bass_kernel_spmd(
        _NC_CACHE, in_maps, core_ids=list(range(NCORES))
    )
    _LAST_RESULTS = res

    out = np.empty((BS, NJOINTS, NFEATS, NFRAMES), np.float32)
    for c in range(NCORES):
        out[B_PER_CORE * c:B_PER_CORE * (c + 1)] = (
            res.results[c]["out"].reshape(B_PER_CORE, NJOINTS, NFEATS, NFRAMES)
        )
    return out
